# revision 1
# baseline (speedup 1.0000x reference)
"""Trainium2 Bass kernel for AttentionFusionLayer.

B=8 batches -> data-parallel across 8 NeuronCores (no collectives).
Per core: x_h, x_c [2048, 1024];  h=lin(x_h), c=lin(x_c), q=lin(h), k=lin(c),
v=lin(c), S=q@k.T/32, attn=softmax(S), out=lin(attn@v + q).

On-chip layout: activations kept feature-major ("transposed", [F, L]) so the
contraction dim of every linear sits on SBUF partitions.  Scores are computed
transposed (S.T[lk, lq]) so attn@v needs no on-chip transpose; softmax skips
max-subtraction (scores are O(1) by construction) and the denominator Z comes
from a ones-column matmul over exp(S.T).  Normalization by 1/Z and the +q
residual are folded into the final output projection: two PSUM accumulation
groups A=U@Wo.T, B=q@Wo.T+ob, out = A*(1/Z) + B computed in natural layout
where 1/Z is a per-partition scalar.

Compute dtype bf16 (fp32 PSUM accumulation); fp32 matmul runs at 1/4 rate on
TRN2 so bf16 is the right choice for this compute-bound problem.
"""

import os
import sys

sys.path.insert(0, "/opt/trn_rl_repo")
os.environ.setdefault("MYCRO_LOCAL_CACHE", "1")

import numpy as np

P = 128
F = 1024
L = 2048
FT = F // P      # 8 feature tiles
LT = L // P      # 16 sequence tiles
BLK = 512        # lq block size
NB = L // BLK    # 4 blocks
SCALE = 1.0 / 32.0  # 1/sqrt(F)

_STATE = {}


def _build():
    from contextlib import ExitStack

    import concourse.tile as tile
    from concourse import bacc, mybir

    BF = mybir.dt.bfloat16
    F32 = mybir.dt.float32
    AF = mybir.ActivationFunctionType
    ALU = mybir.AluOpType

    nc = bacc.Bacc("TRN2", target_bir_lowering=False, debug=False, num_devices=8)

    xhT = nc.dram_tensor("xhT", [F, L], BF, kind="ExternalInput").ap()
    xcT = nc.dram_tensor("xcT", [F, L], BF, kind="ExternalInput").ap()
    wts = {
        n: nc.dram_tensor(f"w_{n}", [F, F], BF, kind="ExternalInput").ap()
        for n in ("h", "c", "q", "k", "v", "o")
    }
    # per-partition bias columns: [:,0:8]=hb, 8:16=cb, 16:24=qb, 24:32=kb
    bpp = nc.dram_tensor("bpp", [P, 32], F32, kind="ExternalInput").ap()
    brow = nc.dram_tensor("brow", [2, F], BF, kind="ExternalInput").ap()  # vb, ob
    out = nc.dram_tensor("out", [L, F], F32, kind="ExternalOutput").ap()

    with tile.TileContext(nc) as tc, ExitStack() as ctx:
        def mkpool(name, bufs, space="SBUF"):
            return ctx.enter_context(tc.tile_pool(name=name, bufs=bufs, space=space))

        p_res = mkpool("res", 1)
        p_io = mkpool("io", 3)      # [P, FT, BLK] bf16 streaming activation blocks
        p_w = mkpool("w", 2)        # [P, FT, F] bf16 resident weight (per layer)
        p_wst = mkpool("wst", 6)    # [P, BLK] bf16 epilogue stage before DRAM spill
        p_exp = mkpool("exp", 2)    # [P, LT, BLK] bf16 exp(S.T) block
        p_u = mkpool("u", 2)        # [P, FT, BLK] bf16 U.T block
        p_ost = mkpool("ost", 2)    # [P, F] f32 output stage
        p_rz = mkpool("rz", 2)      # small 1/Z tiles
        p_dram = mkpool("dram", 1, space="DRAM")
        p_mm = mkpool("mm", 5, space="PSUM")
        p_z = mkpool("z", 1, space="PSUM")
        p_zt = mkpool("zt", 2, space="PSUM")

        # ---- resident tensors & constants ----
        kT = p_res.tile([P, FT, L], BF, tag="kT")       # k transposed [f, lk]
        vN = p_res.tile([P, LT, F], BF, tag="vN")       # v natural [lk, f]
        bpp_t = p_res.tile([P, 32], F32, tag="bpp")
        nc.sync.dma_start(bpp_t[:], bpp)
        vbr = p_res.tile([1, F], BF, tag="vbr")
        nc.sync.dma_start(vbr[:], brow[0:1, :])
        obr = p_res.tile([1, F], BF, tag="obr")
        nc.sync.dma_start(obr[:], brow[1:2, :])
        ones_r = p_res.tile([1, P], BF, tag="ones_r")
        nc.vector.memset(ones_r[:], 1.0)
        ones_c = p_res.tile([P, 1], BF, tag="ones_c")
        nc.vector.memset(ones_c[:], 1.0)
        one1 = p_res.tile([1, 1], F32, tag="one1")
        nc.vector.memset(one1[:], 1.0)

        # vb broadcast to [P, F] via K=1 matmul (row bcast across partitions)
        vb_bc = p_res.tile([P, F], F32, tag="vb_bc")
        for fb in range(2):
            ps = p_mm.tile([P, BLK], F32, tag="mm")
            nc.tensor.matmul(ps[:], ones_r[:], vbr[:, fb * BLK:(fb + 1) * BLK],
                             start=True, stop=True)
            nc.vector.tensor_copy(vb_bc[:, fb * BLK:(fb + 1) * BLK], ps[:])

        hTd = p_dram.tile([F, L], BF, tag="hTd")
        cTd = p_dram.tile([F, L], BF, tag="cTd")
        qTd = p_dram.tile([F, L], BF, tag="qTd")

        def load_w(name):
            W = p_w.tile([P, FT, F], BF, tag="W")
            for ft in range(FT):
                nc.sync.dma_start(W[:, ft, :], wts[name][ft * P:(ft + 1) * P, :])
            return W

        def load_blk(src, b):
            xb = p_io.tile([P, FT, BLK], BF, tag="io")
            for ft in range(FT):
                nc.sync.dma_start(xb[:, ft, :],
                                  src[ft * P:(ft + 1) * P, b * BLK:(b + 1) * BLK])
            return xb

        def t_layer(src, wname, bias_col, dst_dram=None, dst_tile=None, Wv=None):
            """Transposed-output linear: dst.T[fo, l] = W.T.T @ src.T + b.
            If Wv is given, also emit the v-layer (natural layout) from the
            same streamed source blocks (src must then be c.T)."""
            W = load_w(wname)
            for b in range(NB):
                xb = load_blk(src, b)
                for fo in range(FT):
                    ps = p_mm.tile([P, BLK], F32, tag="mm")
                    for fi in range(FT):
                        nc.tensor.matmul(ps[:], W[:, fi, fo * P:(fo + 1) * P],
                                         xb[:, fi, :],
                                         start=(fi == 0), stop=(fi == FT - 1))
                    bias = bpp_t[:, bias_col + fo:bias_col + fo + 1]
                    if dst_tile is not None:
                        nc.scalar.activation(dst_tile[:, fo, b * BLK:(b + 1) * BLK],
                                             ps[:], AF.Identity, bias=bias)
                    else:
                        st = p_wst.tile([P, BLK], BF, tag="wst")
                        nc.scalar.activation(st[:], ps[:], AF.Identity, bias=bias)
                        nc.sync.dma_start(
                            dst_dram[fo * P:(fo + 1) * P, b * BLK:(b + 1) * BLK],
                            st[:])
                if Wv is not None:
                    # v natural: v[lk, fo] = c.T[:, lk].T @ Wv.T[:, fo] + vb
                    for lt in range(BLK // P):
                        lk = b * (BLK // P) + lt
                        for fb in range(2):
                            ps = p_mm.tile([P, BLK], F32, tag="mm")
                            for fi in range(FT):
                                nc.tensor.matmul(
                                    ps[:], xb[:, fi, lt * P:(lt + 1) * P],
                                    Wv[:, fi, fb * BLK:(fb + 1) * BLK],
                                    start=(fi == 0), stop=(fi == FT - 1))
                            nc.vector.tensor_add(
                                vN[:, lk, fb * BLK:(fb + 1) * BLK], ps[:],
                                vb_bc[:, fb * BLK:(fb + 1) * BLK])

        # ---- phase A: projections ----
        t_layer(xhT, "h", 0, dst_dram=hTd)
        t_layer(xcT, "c", 8, dst_dram=cTd)
        t_layer(hTd, "q", 16, dst_dram=qTd)
        Wv = load_w("v")
        t_layer(cTd, "k", 24, dst_tile=kT, Wv=Wv)

        # ---- phase B: attention + output projection ----
        Wo = load_w("o")
        for b in range(NB):
            qb = load_blk(qTd, b)
            # S.T[lk, lq-blk] then exp -> eb
            eb = p_exp.tile([P, LT, BLK], BF, tag="exp")
            for lk in range(LT):
                ps = p_mm.tile([P, BLK], F32, tag="mm")
                for ft in range(FT):
                    nc.tensor.matmul(ps[:], kT[:, ft, lk * P:(lk + 1) * P],
                                     qb[:, ft, :],
                                     start=(ft == 0), stop=(ft == FT - 1))
                nc.scalar.activation(eb[:, lk, :], ps[:], AF.Exp, scale=SCALE)
            # Z[lq] = sum_lk exp(S.T)
            zp = p_z.tile([1, BLK], F32, tag="z")
            for lk in range(LT):
                nc.tensor.matmul(zp[:], ones_c[:], eb[:, lk, :],
                                 start=(lk == 0), stop=(lk == LT - 1))
            rz = p_rz.tile([1, BLK], F32, tag="rz")
            nc.vector.reciprocal(rz[:], zp[:])
            # transpose 1/Z row -> per-partition cols via K=1 matmuls
            rzt = p_rz.tile([P, NB], F32, tag="rzt")
            for ls in range(BLK // P):
                zt = p_zt.tile([P, 1], F32, tag="zt")
                nc.tensor.matmul(zt[:], rz[:, ls * P:(ls + 1) * P], one1[:],
                                 start=True, stop=True)
                nc.vector.tensor_copy(rzt[:, ls:ls + 1], zt[:])
            # U.T[f, lq-blk] = v.T @ exp(S.T)  (unnormalized attn out)
            ub = p_u.tile([P, FT, BLK], BF, tag="u")
            for ft in range(FT):
                ps = p_mm.tile([P, BLK], F32, tag="mm")
                for lk in range(LT):
                    nc.tensor.matmul(ps[:], vN[:, lk, ft * P:(ft + 1) * P],
                                     eb[:, lk, :],
                                     start=(lk == 0), stop=(lk == LT - 1))
                nc.vector.tensor_copy(ub[:, ft, :], ps[:])
            # final: out[lq, fo] = (U.T/Z + q.T).T @ Wo.T + ob, natural layout
            for ls in range(BLK // P):
                ost = p_ost.tile([P, F], F32, tag="ost")
                for fb in range(2):
                    psA = p_mm.tile([P, BLK], F32, tag="mm")
                    for fi in range(FT):
                        nc.tensor.matmul(psA[:], ub[:, fi, ls * P:(ls + 1) * P],
                                         Wo[:, fi, fb * BLK:(fb + 1) * BLK],
                                         start=(fi == 0), stop=(fi == FT - 1))
                    psB = p_mm.tile([P, BLK], F32, tag="mm")
                    for fi in range(FT):
                        nc.tensor.matmul(psB[:], qb[:, fi, ls * P:(ls + 1) * P],
                                         Wo[:, fi, fb * BLK:(fb + 1) * BLK],
                                         start=(fi == 0), stop=False)
                    nc.tensor.matmul(psB[:], ones_r[:],
                                     obr[:, fb * BLK:(fb + 1) * BLK],
                                     start=False, stop=True)
                    sl = slice(fb * BLK, (fb + 1) * BLK)
                    nc.vector.tensor_scalar(ost[:, sl], psA[:],
                                            rzt[:, ls:ls + 1], None, ALU.mult)
                    nc.vector.tensor_add(ost[:, sl], ost[:, sl], psB[:])
                row = (b * (BLK // P) + ls) * P
                nc.sync.dma_start(out[row:row + P, :], ost[:])

    nc.compile()
    return nc


def _get_graph():
    if "nc" not in _STATE:
        _STATE["nc"] = _build()
    return _STATE["nc"]


def kernel(history_features, combined_features, hW, hb, cW, cb, qW, qb, kW, kb,
           vW, vb, oW, ob):
    import ml_dtypes

    from concourse.bass_utils import run_bass_kernel_spmd

    bf16 = ml_dtypes.bfloat16
    B = history_features.shape[0]
    assert B == 8, f"expected batch 8, got {B}"

    nc = _get_graph()

    shared = {
        f"w_{n}": np.ascontiguousarray(np.asarray(W, np.float32).T).astype(bf16)
        for n, W in (("h", hW), ("c", cW), ("q", qW), ("k", kW), ("v", vW),
                     ("o", oW))
    }
    shared["bpp"] = np.concatenate(
        [np.asarray(x, np.float32).reshape(FT, P).T for x in (hb, cb, qb, kb)],
        axis=1)
    shared["brow"] = np.stack([np.asarray(vb, np.float32),
                               np.asarray(ob, np.float32)]).astype(bf16)

    in_maps = []
    for i in range(B):
        m = dict(shared)
        m["xhT"] = np.ascontiguousarray(
            np.asarray(history_features[i], np.float32).T).astype(bf16)
        m["xcT"] = np.ascontiguousarray(
            np.asarray(combined_features[i], np.float32).T).astype(bf16)
        in_maps.append(m)

    res = run_bass_kernel_spmd(nc, in_maps, core_ids=list(range(B)))
    return np.stack([res.results[i]["out"] for i in range(B)]).astype(np.float32)


# revision 3
# speedup vs baseline: 1.2638x; 1.2638x over previous
"""Trainium2 Bass kernel for AttentionFusionLayer.

B=8 batches -> data-parallel across 8 NeuronCores (no collectives).
Per core: x_h, x_c [2048, 1024];  h=lin(x_h), c=lin(x_c), q=lin(h), k=lin(c),
v=lin(c), S=q@k.T/32, attn=softmax(S), out=lin(attn@v + q).

On-chip layout: activations kept feature-major ("transposed", [F, L]) so the
contraction dim of every linear sits on SBUF partitions.  Scores are computed
transposed (S.T[lk, lq]) so attn@v needs no on-chip transpose; softmax skips
max-subtraction (scores are O(1) by construction) and the denominator Z comes
from a ones-column matmul over exp(S.T).  Normalization by 1/Z and the +q
residual are folded into the final output projection: two PSUM accumulation
groups A=U@Wo.T, B=q@Wo.T, out = A*(1/Z) + B + ob computed in natural layout
where 1/Z is a per-partition scalar (Z row is PE-transposed before the
reciprocal so the reciprocal runs wide across lanes).

Compute dtype bf16 (fp32 PSUM accumulation); fp32 matmul runs at 1/4 rate on
TRN2 so bf16 is the right choice for this compute-bound problem.
"""

import os
import sys

sys.path.insert(0, "/opt/trn_rl_repo")
os.environ.setdefault("MYCRO_LOCAL_CACHE", "1")

import numpy as np

P = 128
F = 1024
L = 2048
FT = F // P      # 8 feature tiles
LT = L // P      # 16 sequence tiles
BLK = 512        # lq block size
NB = L // BLK    # 4 blocks
LB = BLK // P    # 4 lq tiles per block
SCALE = 1.0 / 32.0  # 1/sqrt(F)

_STATE = {}


def _build():
    from contextlib import ExitStack

    import concourse.tile as tile
    from concourse import bacc, mybir

    BF = mybir.dt.bfloat16
    F32 = mybir.dt.float32
    AF = mybir.ActivationFunctionType
    ALU = mybir.AluOpType

    nc = bacc.Bacc("TRN2", target_bir_lowering=False, debug=False, num_devices=8)

    xhT = nc.dram_tensor("xhT", [F, L], BF, kind="ExternalInput").ap()
    xcT = nc.dram_tensor("xcT", [F, L], BF, kind="ExternalInput").ap()
    wts = {
        n: nc.dram_tensor(f"w_{n}", [F, F], BF, kind="ExternalInput").ap()
        for n in ("h", "c", "q", "k", "v", "o")
    }
    # per-partition bias columns: [:,0:8]=hb, 8:16=cb, 16:24=qb, 24:32=kb
    bpp = nc.dram_tensor("bpp", [P, 32], F32, kind="ExternalInput").ap()
    brow = nc.dram_tensor("brow", [2, F], BF, kind="ExternalInput").ap()  # vb, ob
    out = nc.dram_tensor("out", [L, F], F32, kind="ExternalOutput").ap()

    with tile.TileContext(nc) as tc, ExitStack() as ctx:
        def mkpool(name, bufs, space="SBUF"):
            return ctx.enter_context(tc.tile_pool(name=name, bufs=bufs, space=space))

        p_res = mkpool("res", 1)
        p_io = mkpool("io", 5)      # [P, FT, BLK] bf16 streaming activation blocks
        p_w = mkpool("w", 2)        # [P, FT, F] bf16 resident weight (per layer)
        p_wst = mkpool("wst", 4)    # [P, BLK] bf16 epilogue stage before DRAM spill
        p_exp = mkpool("exp", 2)    # [P, LT, BLK] bf16 exp(S.T) block
        p_u = mkpool("u", 2)        # [P, FT, BLK] bf16 U.T block
        p_ost = mkpool("ost", 2)    # [P, F] f32 output stage
        p_rz = mkpool("rz", 2)      # small 1/Z tiles
        p_dram = mkpool("dram", 1, space="DRAM")
        p_mm = mkpool("mm", 6, space="PSUM")
        p_z = mkpool("z", 1, space="PSUM")
        p_zt = mkpool("zt", 1, space="PSUM")

        # ---- resident tensors & constants ----
        kT = p_res.tile([P, FT, L], BF, tag="kT")       # k transposed [f, lk]
        vN = p_res.tile([P, LT, F], BF, tag="vN")       # v natural [lk, f]
        bpp_t = p_res.tile([P, 32], F32, tag="bpp")
        nc.sync.dma_start(bpp_t[:], bpp)
        vbr = p_res.tile([1, F], BF, tag="vbr")
        nc.sync.dma_start(vbr[:], brow[0:1, :])
        obr = p_res.tile([1, F], BF, tag="obr")
        nc.sync.dma_start(obr[:], brow[1:2, :])
        ones_r = p_res.tile([1, P], BF, tag="ones_r")
        nc.vector.memset(ones_r[:], 1.0)
        ones_c = p_res.tile([P, 1], BF, tag="ones_c")
        nc.vector.memset(ones_c[:], 1.0)
        one1 = p_res.tile([1, 1], BF, tag="one1")
        nc.vector.memset(one1[:], 1.0)

        # PE warmup: ~5us of tiny matmuls so HAM un-throttles while the first
        # weight/activation DMAs are in flight.
        wrm = p_res.tile([P, P], BF, tag="wrm")
        nc.vector.memset(wrm[:], 0.0)
        wps = p_zt.tile([P, P], F32, tag="zt", name="warm_ps")
        for i in range(48):
            nc.tensor.matmul(wps[:], wrm[:], wrm[:], start=(i == 0),
                             stop=(i == 47), skip_group_check=True)

        # vb/ob broadcast to [P, F] via K=1 matmul (row bcast across partitions)
        vb_bc = p_res.tile([P, F], BF, tag="vb_bc")
        ob_bc = p_res.tile([P, F], BF, tag="ob_bc")
        for bc_dst, bc_src in ((vb_bc, vbr), (ob_bc, obr)):
            for fb in range(2):
                ps = p_mm.tile([P, BLK], F32, tag="mm", name="bc_ps")
                nc.tensor.matmul(ps[:], ones_r[:], bc_src[:, fb * BLK:(fb + 1) * BLK],
                                 start=True, stop=True)
                nc.vector.tensor_copy(bc_dst[:, fb * BLK:(fb + 1) * BLK], ps[:])

        # per-block DRAM spill tiles (finer dependency granularity)
        hTd = [p_dram.tile([F, BLK], BF, tag="hTd", name=f"hTd{b}") for b in range(NB)]
        cTd = [p_dram.tile([F, BLK], BF, tag="cTd", name=f"cTd{b}") for b in range(NB)]
        qTd = [p_dram.tile([F, BLK], BF, tag="qTd", name=f"qTd{b}") for b in range(NB)]

        def load_w(name):
            W = p_w.tile([P, FT, F], BF, tag="W", name=f"W_{name}")
            for ft in range(FT):
                nc.sync.dma_start(W[:, ft, :], wts[name][ft * P:(ft + 1) * P, :])
            return W

        def load_blk(src_blocks, b, nm):
            """src_blocks: list of per-block [F, BLK] dram tiles, or a full
            [F, L] dram AP."""
            xb = p_io.tile([P, FT, BLK], BF, tag="io", name=nm)
            for ft in range(FT):
                if isinstance(src_blocks, list):
                    nc.sync.dma_start(xb[:, ft, :],
                                      src_blocks[b][ft * P:(ft + 1) * P, :])
                else:
                    nc.sync.dma_start(
                        xb[:, ft, :],
                        src_blocks[ft * P:(ft + 1) * P, b * BLK:(b + 1) * BLK])
            return xb

        def t_layer(src, wname, bias_col, dst_dram=None, dst_tile=None,
                    with_v=False):
            """Transposed-output linear: dst.T[fo, l] = W @ src.T + b.
            If with_v, also emit the v-layer (natural layout) from the same
            streamed source blocks (src must then be c.T)."""
            W = load_w(wname)
            Wv = load_w("v") if with_v else None
            for b in range(NB):
                xb = load_blk(src, b, f"x_{wname}{b}")
                for fo in range(FT):
                    ps = p_mm.tile([P, BLK], F32, tag="mm", name=f"ps_{wname}")
                    for fi in range(FT):
                        nc.tensor.matmul(ps[:], W[:, fi, fo * P:(fo + 1) * P],
                                         xb[:, fi, :],
                                         start=(fi == 0), stop=(fi == FT - 1))
                    bias = bpp_t[:, bias_col + fo:bias_col + fo + 1]
                    if dst_tile is not None:
                        nc.scalar.activation(dst_tile[:, fo, b * BLK:(b + 1) * BLK],
                                             ps[:], AF.Identity, bias=bias)
                    else:
                        st = p_wst.tile([P, BLK], BF, tag="wst", name=f"st_{wname}")
                        nc.scalar.activation(st[:], ps[:], AF.Identity, bias=bias)
                        nc.sync.dma_start(dst_dram[b][fo * P:(fo + 1) * P, :], st[:])
                if with_v:
                    # v natural: v[lk, fo] = c.T[:, lk].T @ Wv + vb
                    for lt in range(LB):
                        lk = b * LB + lt
                        for fb in range(2):
                            ps = p_mm.tile([P, BLK], F32, tag="mm", name="ps_v")
                            for fi in range(FT):
                                nc.tensor.matmul(
                                    ps[:], xb[:, fi, lt * P:(lt + 1) * P],
                                    Wv[:, fi, fb * BLK:(fb + 1) * BLK],
                                    start=(fi == 0), stop=(fi == FT - 1))
                            nc.vector.tensor_add(
                                vN[:, lk, fb * BLK:(fb + 1) * BLK], ps[:],
                                vb_bc[:, fb * BLK:(fb + 1) * BLK])

        # ---- phase A: projections ----
        t_layer(xhT, "h", 0, dst_dram=hTd)
        t_layer(xcT, "c", 8, dst_dram=cTd)
        t_layer(hTd, "q", 16, dst_dram=qTd)
        t_layer(cTd, "k", 24, dst_tile=kT, with_v=True)

        # ---- phase B: attention + output projection ----
        Wo = load_w("o")
        for b in range(NB):
            qb = load_blk(qTd, b, f"qb{b}")
            # S.T[lk, lq-blk] then exp -> eb
            eb = p_exp.tile([P, LT, BLK], BF, tag="exp", name=f"eb{b}")
            for lk in range(LT):
                ps = p_mm.tile([P, BLK], F32, tag="mm", name="ps_s")
                for ft in range(FT):
                    nc.tensor.matmul(ps[:], kT[:, ft, lk * P:(lk + 1) * P],
                                     qb[:, ft, :],
                                     start=(ft == 0), stop=(ft == FT - 1))
                nc.scalar.activation(eb[:, lk, :], ps[:], AF.Exp, scale=SCALE)
            # Z[lq] = sum_lk exp(S.T)  (PE: ones-column matmul)
            zp = p_z.tile([1, BLK], F32, tag="z", name=f"zp{b}")
            for lk in range(LT):
                nc.tensor.matmul(zp[:], ones_c[:], eb[:, lk, :],
                                 start=(lk == 0), stop=(lk == LT - 1))
            zrow = p_rz.tile([1, BLK], BF, tag="zrow", name=f"zrow{b}")
            nc.scalar.activation(zrow[:], zp[:], AF.Copy)
            # U.T[f, lq-blk] = v.T @ exp(S.T)  (unnormalized attn out)
            ub = p_u.tile([P, FT, BLK], BF, tag="u", name=f"ub{b}")
            for ft in range(FT):
                ps = p_mm.tile([P, BLK], F32, tag="mm", name="ps_u")
                for lk in range(LT):
                    nc.tensor.matmul(ps[:], vN[:, lk, ft * P:(ft + 1) * P],
                                     eb[:, lk, :],
                                     start=(lk == 0), stop=(lk == LT - 1))
                nc.vector.tensor_copy(ub[:, ft, :], ps[:])
            # transpose Z row into per-partition cols, then wide reciprocal.
            # (emitted after U so the PE has U-work while zrow settles)
            ztp = p_zt.tile([P, LB], F32, tag="zt", name=f"ztp{b}")
            for ls in range(LB):
                nc.tensor.matmul(ztp[:, ls:ls + 1], zrow[:, ls * P:(ls + 1) * P],
                                 one1[:], start=True, stop=True)
            rzt = p_rz.tile([P, LB], F32, tag="rzt", name=f"rzt{b}")
            nc.vector.reciprocal(rzt[:], ztp[:])
            # final: out[lq, fo] = (U.T/Z + q.T).T @ Wo + ob, natural layout
            for ls in range(LB):
                ost = p_ost.tile([P, F], F32, tag="ost", name=f"ost{b}_{ls}")
                for fb in range(2):
                    psA = p_mm.tile([P, BLK], F32, tag="mm", name="ps_fa")
                    for fi in range(FT):
                        nc.tensor.matmul(psA[:], ub[:, fi, ls * P:(ls + 1) * P],
                                         Wo[:, fi, fb * BLK:(fb + 1) * BLK],
                                         start=(fi == 0), stop=(fi == FT - 1))
                    psB = p_mm.tile([P, BLK], F32, tag="mm", name="ps_fb")
                    for fi in range(FT):
                        nc.tensor.matmul(psB[:], qb[:, fi, ls * P:(ls + 1) * P],
                                         Wo[:, fi, fb * BLK:(fb + 1) * BLK],
                                         start=(fi == 0), stop=(fi == FT - 1))
                    sl = slice(fb * BLK, (fb + 1) * BLK)
                    nc.vector.tensor_scalar(ost[:, sl], psA[:],
                                            rzt[:, ls:ls + 1], None, ALU.mult)
                    nc.vector.tensor_add(ost[:, sl], ost[:, sl], psB[:])
                    nc.gpsimd.tensor_add(ost[:, sl], ost[:, sl], ob_bc[:, sl])
                row = (b * LB + ls) * P
                nc.sync.dma_start(out[row:row + P, :], ost[:])

    nc.compile()
    return nc


def _get_graph():
    if "nc" not in _STATE:
        _STATE["nc"] = _build()
    return _STATE["nc"]


def kernel(history_features, combined_features, hW, hb, cW, cb, qW, qb, kW, kb,
           vW, vb, oW, ob):
    import ml_dtypes

    from concourse.bass_utils import run_bass_kernel_spmd

    bf16 = ml_dtypes.bfloat16
    B = history_features.shape[0]
    assert B == 8, f"expected batch 8, got {B}"

    nc = _get_graph()

    shared = {
        f"w_{n}": np.ascontiguousarray(np.asarray(W, np.float32).T).astype(bf16)
        for n, W in (("h", hW), ("c", cW), ("q", qW), ("k", kW), ("v", vW),
                     ("o", oW))
    }
    shared["bpp"] = np.concatenate(
        [np.asarray(x, np.float32).reshape(FT, P).T for x in (hb, cb, qb, kb)],
        axis=1)
    shared["brow"] = np.stack([np.asarray(vb, np.float32),
                               np.asarray(ob, np.float32)]).astype(bf16)

    in_maps = []
    for i in range(B):
        m = dict(shared)
        m["xhT"] = np.ascontiguousarray(
            np.asarray(history_features[i], np.float32).T).astype(bf16)
        m["xcT"] = np.ascontiguousarray(
            np.asarray(combined_features[i], np.float32).T).astype(bf16)
        in_maps.append(m)

    res = run_bass_kernel_spmd(nc, in_maps, core_ids=list(range(B)))
    return np.stack([res.results[i]["out"] for i in range(B)]).astype(np.float32)


# revision 4
# speedup vs baseline: 1.5194x; 1.2023x over previous
"""Trainium2 Bass kernel for AttentionFusionLayer.

B=8 batches -> data-parallel across 8 NeuronCores (no collectives).
Per core: x_h, x_c [2048, 1024];  h=lin(x_h), c=lin(x_c), q=lin(h), k=lin(c),
v=lin(c), S=q@k.T/32, attn=softmax(S), out=lin(attn@v + q).

On-chip layout: activations kept feature-major ("transposed", [F, L]) so the
contraction dim of every linear sits on SBUF partitions.  Scores are computed
transposed (S.T[lk, lq]) so attn@v needs no on-chip transpose; softmax skips
max-subtraction (scores are O(1) by construction) and the denominator Z comes
from a ones-column matmul over exp(S.T).  Normalization by 1/Z and the +q
residual are folded into the final output projection: two PSUM accumulation
groups A=U@Wo.T, B=q@Wo.T, out = A*(1/Z) + B + ob computed in natural layout
where 1/Z is a per-partition scalar.

Precision split: the residual-dominant path (h and q projections, B=q@Wo)
stays bf16; everything whose error only enters through the attention output
(~4% of output magnitude) runs fp8-e4m3 with DoubleRow matmuls (2 MACs per
cell per cycle): the c/k/v projections, S=q@k.T, U=expS@v and A=U@Wo.
All accumulation is fp32 in PSUM.
"""

import os
import sys

sys.path.insert(0, "/opt/trn_rl_repo")
os.environ.setdefault("MYCRO_LOCAL_CACHE", "1")

import numpy as np

P = 128
F = 1024
L = 2048
FT = F // P      # 8 feature tiles
FP = FT // 2     # 4 feature-tile pairs (DoubleRow)
LT = L // P      # 16 sequence tiles
BLK = 512        # lq block size
NB = L // BLK    # 4 blocks
LB = BLK // P    # 4 lq tiles per block
SCALE = 1.0 / 32.0  # 1/sqrt(F)

_STATE = {}


def _build():
    from contextlib import ExitStack

    import concourse.tile as tile
    from concourse import bacc, mybir

    BF = mybir.dt.bfloat16
    F8 = mybir.dt.float8e4
    F32 = mybir.dt.float32
    AF = mybir.ActivationFunctionType
    ALU = mybir.AluOpType
    DR = mybir.MatmulPerfMode.DoubleRow

    nc = bacc.Bacc("TRN2", target_bir_lowering=False, debug=False, num_devices=8)

    xhT = nc.dram_tensor("xhT", [F, L], BF, kind="ExternalInput").ap()
    xcT = nc.dram_tensor("xcT", [F, L], F8, kind="ExternalInput").ap()
    wts = {}
    for n, dt_ in (("h", BF), ("c", F8), ("q", BF), ("k", F8), ("v", F8),
                   ("o", BF), ("o8", F8)):
        wts[n] = nc.dram_tensor(f"w_{n}", [F, F], dt_, kind="ExternalInput").ap()
    # per-partition bias columns: [:,0:8]=hb, 8:16=cb, 16:24=qb, 24:32=kb
    bpp = nc.dram_tensor("bpp", [P, 32], F32, kind="ExternalInput").ap()
    brow = nc.dram_tensor("brow", [2, F], BF, kind="ExternalInput").ap()  # vb, ob
    out = nc.dram_tensor("out", [L, F], F32, kind="ExternalOutput").ap()

    with tile.TileContext(nc) as tc, ExitStack() as ctx:
        def mkpool(name, bufs, space="SBUF"):
            return ctx.enter_context(tc.tile_pool(name=name, bufs=bufs, space=space))

        p_res = mkpool("res", 1)
        p_io = mkpool("io", 8)      # [P, FT, BLK] streaming activation blocks
        p_w = mkpool("w", 2)        # [P, FT, F] resident weight (per layer)
        p_wst = mkpool("wst", 6)    # [P, BLK] epilogue stage before DRAM spill
        p_exp = mkpool("exp", 2)    # [P, LT, BLK] f8 exp(S.T) block
        p_u = mkpool("u", 2)        # [P, FT, BLK] f8 U.T block
        p_q8 = mkpool("q8", 2)      # [P, FT, BLK] f8 q block
        p_ost = mkpool("ost", 2)    # [P, F] f32 output stage
        p_rz = mkpool("rz", 2)      # small 1/Z tiles
        p_dram = mkpool("dram", 1, space="DRAM")
        p_mm = mkpool("mm", 6, space="PSUM")
        p_z = mkpool("z", 1, space="PSUM")
        p_zt = mkpool("zt", 1, space="PSUM")

        # ---- resident tensors & constants ----
        kT = p_res.tile([P, FT, L], F8, tag="kT")       # k transposed [f, lk]
        vN = p_res.tile([P, LT, F], F8, tag="vN")       # v natural [lk, f]
        bpp_t = p_res.tile([P, 32], F32, tag="bpp")
        nc.sync.dma_start(bpp_t[:], bpp)
        vbr = p_res.tile([1, F], BF, tag="vbr")
        nc.sync.dma_start(vbr[:], brow[0:1, :])
        obr = p_res.tile([1, F], BF, tag="obr")
        nc.sync.dma_start(obr[:], brow[1:2, :])
        ones_r = p_res.tile([1, P], BF, tag="ones_r")
        nc.vector.memset(ones_r[:], 1.0)
        ones_c8 = p_res.tile([P, 1], F8, tag="ones_c8")
        nc.vector.memset(ones_c8[:], 1.0)
        one1 = p_res.tile([1, 1], BF, tag="one1")
        nc.vector.memset(one1[:], 1.0)

        # PE warmup: tiny matmuls so HAM un-throttles while the first
        # weight/activation DMAs are in flight.
        wrm = p_res.tile([P, P], BF, tag="wrm")
        nc.vector.memset(wrm[:], 0.0)
        wps = p_zt.tile([P, P], F32, tag="zt", name="warm_ps")
        for i in range(72):
            nc.tensor.matmul(wps[:], wrm[:], wrm[:], start=(i == 0),
                             stop=(i == 71), skip_group_check=True)

        # vb/ob broadcast to [P, F] via K=1 matmul (row bcast across partitions)
        vb_bc = p_res.tile([P, F], BF, tag="vb_bc")
        ob_bc = p_res.tile([P, F], BF, tag="ob_bc")
        for bc_dst, bc_src in ((vb_bc, vbr), (ob_bc, obr)):
            for fb in range(2):
                ps = p_mm.tile([P, BLK], F32, tag="mm", name="bc_ps")
                nc.tensor.matmul(ps[:], ones_r[:], bc_src[:, fb * BLK:(fb + 1) * BLK],
                                 start=True, stop=True)
                nc.vector.tensor_copy(bc_dst[:, fb * BLK:(fb + 1) * BLK], ps[:])

        # per-block DRAM spill tiles (finer dependency granularity)
        hTd = [p_dram.tile([F, BLK], BF, tag="hTd", name=f"hTd{b}") for b in range(NB)]
        cTd = [p_dram.tile([F, BLK], F8, tag="cTd", name=f"cTd{b}") for b in range(NB)]
        qTd = [p_dram.tile([F, BLK], BF, tag="qTd", name=f"qTd{b}") for b in range(NB)]

        def load_w(name, dt_):
            W = p_w.tile([P, FT, F], dt_, tag="W", name=f"W_{name}")
            for ft in range(FT):
                nc.sync.dma_start(W[:, ft, :], wts[name][ft * P:(ft + 1) * P, :])
            return W

        def load_blk(src_blocks, b, nm, dt_):
            xb = p_io.tile([P, FT, BLK], dt_, tag="io", name=nm)
            for ft in range(FT):
                if isinstance(src_blocks, list):
                    nc.sync.dma_start(xb[:, ft, :],
                                      src_blocks[b][ft * P:(ft + 1) * P, :])
                else:
                    nc.sync.dma_start(
                        xb[:, ft, :],
                        src_blocks[ft * P:(ft + 1) * P, b * BLK:(b + 1) * BLK])
            return xb

        def t_layer(src, wname, bias_col, src_dt, out_dt, dst_dram=None,
                    dst_tile=None, with_v=False, f8=False):
            """Transposed-output linear: dst.T[fo, l] = W @ src.T + b.
            f8: fp8 DoubleRow matmuls (both operands must be fp8).
            If with_v, also emit the v-layer (natural layout) from the same
            streamed source blocks (src must then be c.T)."""
            W = load_w(wname, F8 if f8 else BF)
            Wv = load_w("v", F8) if with_v else None
            for b in range(NB):
                xb = load_blk(src, b, f"x_{wname}{b}", src_dt)
                for fo in range(FT):
                    ps = p_mm.tile([P, BLK], F32, tag="mm", name=f"ps_{wname}")
                    if f8:
                        for t in range(FP):
                            nc.tensor.matmul(
                                ps[:], W[:, 2 * t:2 * t + 2, fo * P:(fo + 1) * P],
                                xb[:, 2 * t:2 * t + 2, :], perf_mode=DR,
                                start=(t == 0), stop=(t == FP - 1))
                    else:
                        for fi in range(FT):
                            nc.tensor.matmul(ps[:], W[:, fi, fo * P:(fo + 1) * P],
                                             xb[:, fi, :],
                                             start=(fi == 0), stop=(fi == FT - 1))
                    bias = bpp_t[:, bias_col + fo:bias_col + fo + 1]
                    if dst_tile is not None:
                        nc.scalar.activation(dst_tile[:, fo, b * BLK:(b + 1) * BLK],
                                             ps[:], AF.Identity, bias=bias)
                    else:
                        st = p_wst.tile([P, BLK], out_dt, tag="wst",
                                        name=f"st_{wname}")
                        nc.scalar.activation(st[:], ps[:], AF.Identity, bias=bias)
                        nc.sync.dma_start(dst_dram[b][fo * P:(fo + 1) * P, :], st[:])
                if with_v:
                    # v natural: v[lk, fo] = c.T[:, lk].T @ Wv + vb
                    for lt in range(LB):
                        lk = b * LB + lt
                        for fb in range(2):
                            ps = p_mm.tile([P, BLK], F32, tag="mm", name="ps_v")
                            for t in range(FP):
                                nc.tensor.matmul(
                                    ps[:],
                                    xb[:, 2 * t:2 * t + 2, lt * P:(lt + 1) * P],
                                    Wv[:, 2 * t:2 * t + 2, fb * BLK:(fb + 1) * BLK],
                                    perf_mode=DR,
                                    start=(t == 0), stop=(t == FP - 1))
                            nc.vector.tensor_add(
                                vN[:, lk, fb * BLK:(fb + 1) * BLK], ps[:],
                                vb_bc[:, fb * BLK:(fb + 1) * BLK])

        # ---- phase A: projections ----
        t_layer(xhT, "h", 0, BF, BF, dst_dram=hTd)
        t_layer(xcT, "c", 8, F8, F8, dst_dram=cTd, f8=True)
        t_layer(hTd, "q", 16, BF, BF, dst_dram=qTd)
        t_layer(cTd, "k", 24, F8, F8, dst_tile=kT, with_v=True, f8=True)

        # ---- phase B: attention + output projection ----
        Wo = load_w("o", BF)
        Wo8 = load_w("o8", F8)
        for b in range(NB):
            qb = load_blk(qTd, b, f"qb{b}", BF)
            qb8 = p_q8.tile([P, FT, BLK], F8, tag="q8", name=f"qb8_{b}")
            nc.scalar.activation(qb8[:], qb[:], AF.Copy)
            # S.T[lk, lq-blk] then exp -> eb (fp8)
            eb = p_exp.tile([P, LT, BLK], F8, tag="exp", name=f"eb{b}")
            for lk in range(LT):
                ps = p_mm.tile([P, BLK], F32, tag="mm", name="ps_s")
                for t in range(FP):
                    nc.tensor.matmul(ps[:],
                                     kT[:, 2 * t:2 * t + 2, lk * P:(lk + 1) * P],
                                     qb8[:, 2 * t:2 * t + 2, :], perf_mode=DR,
                                     start=(t == 0), stop=(t == FP - 1))
                nc.scalar.activation(eb[:, lk, :], ps[:], AF.Exp, scale=SCALE)
            # Z[lq] = sum_lk exp(S.T)  (ones-column fp8 matmuls, normal mode)
            zp = p_z.tile([1, BLK], F32, tag="z", name=f"zp{b}")
            for lk in range(LT):
                nc.tensor.matmul(zp[:], ones_c8[:], eb[:, lk, :],
                                 start=(lk == 0), stop=(lk == LT - 1))
            zrow = p_rz.tile([1, BLK], BF, tag="zrow", name=f"zrow{b}")
            nc.scalar.activation(zrow[:], zp[:], AF.Copy)
            # U.T[f, lq-blk] = v.T @ exp(S.T)  (unnormalized attn out, f8 DR)
            ub = p_u.tile([P, FT, BLK], F8, tag="u", name=f"ub{b}")
            for ft in range(FT):
                ps = p_mm.tile([P, BLK], F32, tag="mm", name="ps_u")
                for t in range(LT // 2):
                    nc.tensor.matmul(ps[:],
                                     vN[:, 2 * t:2 * t + 2, ft * P:(ft + 1) * P],
                                     eb[:, 2 * t:2 * t + 2, :], perf_mode=DR,
                                     start=(t == 0), stop=(t == LT // 2 - 1))
                nc.vector.tensor_copy(ub[:, ft, :], ps[:])
            # transpose Z row into per-partition cols, then wide reciprocal.
            # (emitted after U so the PE has U-work while zrow settles)
            ztp = p_zt.tile([P, LB], F32, tag="zt", name=f"ztp{b}")
            for ls in range(LB):
                nc.tensor.matmul(ztp[:, ls:ls + 1], zrow[:, ls * P:(ls + 1) * P],
                                 one1[:], start=True, stop=True)
            rzt = p_rz.tile([P, LB], F32, tag="rzt", name=f"rzt{b}")
            nc.vector.reciprocal(rzt[:], ztp[:])
            # final: out[lq, fo] = (U.T/Z + q.T).T @ Wo + ob, natural layout
            for ls in range(LB):
                ost = p_ost.tile([P, F], F32, tag="ost", name=f"ost{b}_{ls}")
                for fb in range(2):
                    psA = p_mm.tile([P, BLK], F32, tag="mm", name="ps_fa")
                    for t in range(FP):
                        nc.tensor.matmul(
                            psA[:], ub[:, 2 * t:2 * t + 2, ls * P:(ls + 1) * P],
                            Wo8[:, 2 * t:2 * t + 2, fb * BLK:(fb + 1) * BLK],
                            perf_mode=DR, start=(t == 0), stop=(t == FP - 1))
                    psB = p_mm.tile([P, BLK], F32, tag="mm", name="ps_fb")
                    for fi in range(FT):
                        nc.tensor.matmul(psB[:], qb[:, fi, ls * P:(ls + 1) * P],
                                         Wo[:, fi, fb * BLK:(fb + 1) * BLK],
                                         start=(fi == 0), stop=(fi == FT - 1))
                    sl = slice(fb * BLK, (fb + 1) * BLK)
                    nc.vector.tensor_scalar(ost[:, sl], psA[:],
                                            rzt[:, ls:ls + 1], None, ALU.mult)
                    nc.vector.tensor_add(ost[:, sl], ost[:, sl], psB[:])
                    nc.gpsimd.tensor_add(ost[:, sl], ost[:, sl], ob_bc[:, sl])
                row = (b * LB + ls) * P
                nc.sync.dma_start(out[row:row + P, :], ost[:])

    nc.compile()
    return nc


def _get_graph():
    if "nc" not in _STATE:
        _STATE["nc"] = _build()
    return _STATE["nc"]


def kernel(history_features, combined_features, hW, hb, cW, cb, qW, qb, kW, kb,
           vW, vb, oW, ob):
    import ml_dtypes

    from concourse.bass_utils import run_bass_kernel_spmd

    bf16 = ml_dtypes.bfloat16
    f8 = ml_dtypes.float8_e4m3fn
    B = history_features.shape[0]
    assert B == 8, f"expected batch 8, got {B}"

    nc = _get_graph()

    def wt(W, dt_):
        return np.ascontiguousarray(np.asarray(W, np.float32).T).astype(dt_)

    shared = {
        "w_h": wt(hW, bf16), "w_c": wt(cW, f8), "w_q": wt(qW, bf16),
        "w_k": wt(kW, f8), "w_v": wt(vW, f8), "w_o": wt(oW, bf16),
        "w_o8": wt(oW, f8),
    }
    shared["bpp"] = np.concatenate(
        [np.asarray(x, np.float32).reshape(FT, P).T for x in (hb, cb, qb, kb)],
        axis=1)
    shared["brow"] = np.stack([np.asarray(vb, np.float32),
                               np.asarray(ob, np.float32)]).astype(bf16)

    in_maps = []
    for i in range(B):
        m = dict(shared)
        m["xhT"] = np.ascontiguousarray(
            np.asarray(history_features[i], np.float32).T).astype(bf16)
        m["xcT"] = np.ascontiguousarray(
            np.asarray(combined_features[i], np.float32).T).astype(f8)
        in_maps.append(m)

    res = run_bass_kernel_spmd(nc, in_maps, core_ids=list(range(B)))
    return np.stack([res.results[i]["out"] for i in range(B)]).astype(np.float32)
